# revision 1
# baseline (speedup 1.0000x reference)
"""Expert-parallel MoE FFN kernel for Trainium2 (8 NeuronCores).

Problem: inputs [B=2, E=8, C=8192, H=512], per-expert FFN
    h   = gelu_tanh(x_e @ w1_e + b1_e)        (w1: [E, H, F=2048])
    out = h @ w2_e + b2_e                     (w2: [E, F, H])

Sharding: expert-parallel — core e owns expert e's tokens [B*C, H] and
weights; no cross-core communication.

Per-core dataflow (matmuls in float32r = full-rate fp32 on the PE):
    x tile [128 tok, 512 H]  --PE transpose-->  xT [H-part, tok]
    GEMM1: hT[f,:] += w1[hk, f-chunk].T @ xT[hk, :]   (lhsT = w1, natural layout)
    gelu on ACT during PSUM->SBUF eviction (bias = b1 per-partition)
    GEMM2: out[tok,:] += hT[fk, tok-chunk].T @ w2[fk, :] (lhsT = hT, rhs = w2 natural)
    b2 add on DVE during PSUM->SBUF eviction

FP32r discipline: the BIR verifier requires every producer of an f32r
matmul input to round its output to f32r. xT and hT are produced by
DVE/ACT ops writing into f32r tiles (rounding happens on the write);
weights are DMA'd as raw bytes into f32r tiles ("dma" mode) or staged
through f32 tiles + a rounding DVE copy ("stage" mode).
"""

import numpy as np

_B, _E, _C, _H, _F = 2, 8, 8192, 512, 2048
_TOK = _B * _C  # 16384 tokens per expert
_P = 128
_T = 512  # tokens per macro tile

_MM_MODE = "f32r_dma"  # "f32r_dma" | "f32r_stage" | "f32"

_cache = {}


def build_nc(tok=_TOK, act_name="Gelu_apprx_tanh", n_devices=_E, mm_mode=_MM_MODE,
             loop_reps=1, skip=(), psum_cfg=(2, 4, 2), tp_via_mm=False,
             wide_gelu=False):
    import contextlib

    import concourse.mybir as mybir
    import concourse.tile as tile
    from concourse import bacc
    from concourse.masks import make_identity

    H, F, P, T = _H, _F, _P, _T
    HK, FK = H // P, F // P  # 4, 16 contraction chunks
    NJ = T // P  # 4 token sub-blocks per macro tile
    NM = tok // T  # macro tiles
    if wide_gelu:
        psum_cfg = (2, 2, 2)  # ps1 tiles span 2 banks in this mode
    f32 = mybir.dt.float32
    f32r = mybir.dt.float32r
    mmdt = {"f32": f32, "bf16": mybir.dt.bfloat16}.get(mm_mode, f32r)
    act = getattr(mybir.ActivationFunctionType, act_name)

    nc = bacc.Bacc("TRN2", debug=False, target_bir_lowering=False,
                   num_devices=n_devices)
    wdt = f32r if mm_mode == "f32r_dma" else f32
    xdt = mmdt if tp_via_mm else f32
    x = nc.dram_tensor("x", [tok, H], xdt, kind="ExternalInput").ap()
    w1 = nc.dram_tensor("w1", [H, F], wdt, kind="ExternalInput").ap()
    b1 = nc.dram_tensor("b1", [F], f32, kind="ExternalInput").ap()
    w2 = nc.dram_tensor("w2", [F, H], wdt, kind="ExternalInput").ap()
    b2 = nc.dram_tensor("b2", [H], f32, kind="ExternalInput").ap()
    out = nc.dram_tensor("out", [tok, H], f32, kind="ExternalOutput").ap()

    with tile.TileContext(nc) as tc:
        with (
            tc.tile_pool(name="const", bufs=1) as const,
            tc.tile_pool(name="stage", bufs=1) as stage,
            tc.tile_pool(name="xin", bufs=2) as xin_pool,
            tc.tile_pool(name="xt", bufs=2) as xt_pool,
            tc.tile_pool(name="ht", bufs=2) as ht_pool,
            tc.tile_pool(name="obuf", bufs=1) as o_pool,
            tc.tile_pool(name="scr", bufs=4) as scratch,
            tc.tile_pool(name="pst", bufs=psum_cfg[0], space="PSUM") as psT,
            tc.tile_pool(name="ps1", bufs=psum_cfg[1], space="PSUM") as ps1,
            tc.tile_pool(name="ps2", bufs=psum_cfg[2], space="PSUM") as ps2,
        ):
            # --- weights / constants, resident in SBUF for the whole kernel
            w1_sb = const.tile([P, HK, F], mmdt)
            w2_sb = const.tile([P, FK, H], mmdt)
            if mm_mode == "f32r_stage":
                w1st = stage.tile([P, HK * F], f32, tag="wst")
                nc.sync.dma_start(
                    w1st[:], w1.rearrange("(hk p) f -> p (hk f)", p=P))
                nc.vector.tensor_copy(
                    w1_sb.rearrange("p hk f -> p (hk f)"), w1st[:])
                w2st = stage.tile([P, FK * H], f32, tag="wst")
                nc.sync.dma_start(
                    w2st[:], w2.rearrange("(fk p) h -> p (fk h)", p=P))
                nc.vector.tensor_copy(
                    w2_sb.rearrange("p fk h -> p (fk h)"), w2st[:])
            elif mm_mode == "bf16":
                # SWDGE casts f32 DRAM -> bf16 SBUF during the transfer
                nc.gpsimd.dma_start(
                    w1_sb[:], w1.rearrange("(hk p) f -> p hk f", p=P))
                nc.gpsimd.dma_start(
                    w2_sb[:], w2.rearrange("(fk p) h -> p fk h", p=P))
            else:
                nc.sync.dma_start(
                    w1_sb[:], w1.rearrange("(hk p) f -> p hk f", p=P))
                nc.sync.dma_start(
                    w2_sb[:], w2.rearrange("(fk p) h -> p fk h", p=P))
            b1_sb = const.tile([P, FK], f32)
            nc.sync.dma_start(b1_sb[:], b1.rearrange("(fk p) -> p fk", p=P))
            b2_row = const.tile([1, H], f32)
            nc.sync.dma_start(b2_row[:], b2[None, :])
            ones = const.tile([1, P], f32)
            nc.any.memset(ones[:], 1.0)
            ident = const.tile([P, P], f32)
            make_identity(nc, ident[:])
            ident_r = None
            if tp_via_mm:
                ident_r = const.tile([P, P], mmdt)
                nc.vector.tensor_copy(ident_r[:], ident[:])
            # broadcast b2 across all 128 partitions via a K=1 matmul
            b2_bc = const.tile([P, H], f32)
            ps_b2 = ps2.tile([P, H], f32, tag="po")
            nc.tensor.matmul(ps_b2[:], ones[:], b2_row[:], start=True, stop=True)
            nc.vector.tensor_copy(b2_bc[:], ps_b2[:])

            static_xt = None
            if "transpose" in skip:  # timing-only variant
                static_xt = const.tile([P, HK, T], mmdt, name="static_xt")
                for hk in range(HK):
                    nc.vector.tensor_copy(static_xt[:, hk, :], b2_bc[:])
            static_ht = None
            if "gemm1" in skip:  # timing-only variant
                static_ht = const.tile([P, FK, T], mmdt, name="static_ht")
                for fk in range(FK):
                    nc.vector.tensor_copy(static_ht[:, fk, :], b2_bc[:])

            def prepare(m):
                """DMA-load macro tile m and transpose it into xT.

                Emitted one macro ahead (between GEMM1(m-1) and GEMM2(m-1))
                so the PE transposes slot into the GEMM1->GEMM2 gap and the
                DVE copies overlap GEMM2's PE work.
                """
                r = m * T
                if "indma" not in skip:
                    xbig = xin_pool.tile([P, NJ, H], xdt,
                                         name=f"xbig{m}", tag="xbig")
                    nc.sync.dma_start(
                        xbig[:], x[r:r + T, :].rearrange("(j p) h -> p j h", p=P))
                if "transpose" in skip:
                    return static_xt
                xt = xt_pool.tile([P, HK, T], mmdt, name=f"xt{m}", tag="xt")
                for hk in range(HK):
                    # 4 transposed [128,128] blocks share one PSUM bank, then
                    # a single wide DVE copy evicts them -> far fewer PE<->DVE
                    # round-trips than per-block eviction
                    pt = psT.tile([P, NJ, P], f32)
                    for j in range(NJ):
                        if tp_via_mm:
                            # regular matmul x_blk.T @ I — avoids PE
                            # transpose-mode switches between GEMM groups
                            nc.tensor.matmul(
                                pt[:, j, :], xbig[:, j, hk * P:(hk + 1) * P],
                                ident_r[:], start=True, stop=True)
                        else:
                            nc.tensor.transpose(
                                pt[:, j, :], xbig[:, j, hk * P:(hk + 1) * P],
                                ident[:])
                    nc.vector.tensor_copy(xt[:, hk, :], pt[:])
                return xt

            # loop_reps > 1 wraps the body in a hardware loop — used only by
            # the timing harness to amortize per-dispatch overhead.
            reps_ctx = (tc.For_i(0, loop_reps, 1) if loop_reps > 1
                        else contextlib.nullcontext())
            def g1_group(xt, ht, m, fm):
                """One GEMM1 accumulation group (4 matmuls) + gelu eviction.

                Evictions alternate engines: even fm goes ACT gelu straight
                from PSUM; odd fm is DVE-copied to SBUF scratch first and
                gelu'd from there. Back-to-back ACT PSUM evictions of
                consecutive groups collapse PE throughput ~17x (measured);
                alternating eliminates that.
                """
                ph = ps1.tile([P, T], f32, name=f"ph{m}_{fm}", tag="ph")
                for hk in range(HK):
                    nc.tensor.matmul(
                        ph[:],
                        w1_sb[:, hk, fm * P:(fm + 1) * P],
                        xt[:, hk, :],
                        start=(hk == 0),
                        stop=(hk == HK - 1),
                    )
                if fm % 2 == 0:
                    nc.scalar.activation(
                        ht[:, fm, :], ph[:], act, bias=b1_sb[:, fm:fm + 1])
                else:
                    sc = scratch.tile([P, T], f32, name=f"sc{m}_{fm}", tag="sc")
                    nc.vector.tensor_copy(sc[:], ph[:])
                    nc.scalar.activation(
                        ht[:, fm, :], sc[:], act, bias=b1_sb[:, fm:fm + 1])

            def g1_pair(xt, ht, m, fmp):
                """Two GEMM1 groups into one 2-bank PSUM tile, evicted by a
                single wide [128,1024] gelu. Only valid when b1 == 0 (one
                activation op cannot apply two different per-partition
                biases); kernel() selects this variant at runtime."""
                ph = ps1.tile([P, 2, T], f32, name=f"php{m}_{fmp}", tag="ph")
                for half in range(2):
                    fm = 2 * fmp + half
                    for hk in range(HK):
                        nc.tensor.matmul(
                            ph[:, half, :],
                            w1_sb[:, hk, fm * P:(fm + 1) * P],
                            xt[:, hk, :],
                            start=(hk == 0),
                            stop=(hk == HK - 1),
                        )
                nc.scalar.activation(
                    ht[:, 2 * fmp:2 * fmp + 2, :], ph[:], act, bias=0.0)

            def g2_group(ht, obig, j):
                """One GEMM2 accumulation group (16 matmuls) + b2 eviction."""
                po = ps2.tile([P, H], f32, tag="po", name="po")
                for fk in range(FK):
                    nc.tensor.matmul(
                        po[:],
                        ht[:, fk, j * P:(j + 1) * P],
                        w2_sb[:, fk, :],
                        start=(fk == 0),
                        stop=(fk == FK - 1),
                    )
                nc.vector.tensor_add(obig[:, j, :], po[:], b2_bc[:])

            def new_ht(m):
                return ht_pool.tile([P, FK, T], mmdt, name=f"ht{m}", tag="ht")

            # Software pipeline, one macro ahead on GEMM1:
            #   prepare(m+1) ; G2(m)g0 ; [G1(m+1) x4 ; G2(m) g] x3 ; G1(m+1) x4
            # GEMM1(m+1) interleaves with GEMM2(m) so ACT gelu work spreads
            # over the whole timeline instead of bunching in the GEMM1 phase.
            do_g1 = "gemm1" not in skip
            do_g2 = "gemm2" not in skip
            with reps_ctx:
                xt = prepare(0)
                ht = new_ht(0) if do_g1 else static_ht
                if do_g1:
                    if wide_gelu:
                        for fmp in range(FK // 2):
                            g1_pair(xt, ht, 0, fmp)
                    else:
                        for fm in range(FK):
                            g1_group(xt, ht, 0, fm)
                for m in range(NM):
                    r = m * T
                    if m + 1 < NM:
                        xt = prepare(m + 1)
                        ht_next = new_ht(m + 1) if do_g1 else static_ht
                    else:
                        ht_next = None
                    obig = o_pool.tile([P, NJ, H], f32, name=f"ob{m}", tag="ob")
                    if ht_next is not None and do_g1:
                        if wide_gelu:
                            for fmp in range(FK // 2):
                                g1_pair(xt, ht_next, m + 1, fmp)
                        else:
                            for fm in range(FK):
                                g1_group(xt, ht_next, m + 1, fm)
                    if do_g2:
                        for j in range(NJ):
                            g2_group(ht, obig, j)
                    if "outdma" not in skip and do_g2:
                        nc.sync.dma_start(
                            out[r:r + T, :].rearrange("(j p) h -> p j h", p=P),
                            obig[:])
                    ht = ht_next

    nc.compile()
    return nc


def kernel(inputs, w1, b1, w2, b2):
    from concourse.bass_utils import run_bass_kernel_spmd

    inputs = np.asarray(inputs, dtype=np.float32)
    w1 = np.asarray(w1, dtype=np.float32)
    b1 = np.asarray(b1, dtype=np.float32)
    w2 = np.asarray(w2, dtype=np.float32)
    b2 = np.asarray(b2, dtype=np.float32)

    B, E, C, H = inputs.shape
    tok = B * C
    # [B, E, C, H] -> per-expert token matrix [E, B*C, H]
    x = np.ascontiguousarray(inputs.transpose(1, 0, 2, 3).reshape(E, tok, H))

    if "nc" not in _cache:
        _cache["nc"] = build_nc()
    nc = _cache["nc"]

    in_maps = [
        {
            "x": x[e],
            "w1": np.ascontiguousarray(w1[e]),
            "b1": np.ascontiguousarray(b1[e]),
            "w2": np.ascontiguousarray(w2[e]),
            "b2": np.ascontiguousarray(b2[e]),
        }
        for e in range(E)
    ]
    res = run_bass_kernel_spmd(nc, in_maps, core_ids=list(range(E)))
    o = np.stack([res.results[e]["out"] for e in range(E)])  # [E, tok, H]
    return np.ascontiguousarray(
        o.reshape(E, B, C, H).transpose(1, 0, 2, 3))



# revision 2
# speedup vs baseline: 1.0153x; 1.0153x over previous
"""Expert-parallel MoE FFN kernel for Trainium2 (8 NeuronCores).

Problem: inputs [B=2, E=8, C=8192, H=512], per-expert FFN
    h   = gelu_tanh(x_e @ w1_e + b1_e)        (w1: [E, H, F=2048])
    out = h @ w2_e + b2_e                     (w2: [E, F, H])

Sharding: expert-parallel — core e owns expert e's tokens [B*C, H] and
weights; no cross-core communication.

Per-core dataflow (matmuls in float32r = full-rate fp32 on the PE):
    x is transposed on the HOST to xT [H, tok] so each macro tile loads
    straight into the GEMM1 rhs layout — no PE transposes at all.
    GEMM1: hT[f,:] += w1[hk, f-chunk].T @ xT[hk, :]   (lhsT = w1, natural layout)
    gelu on ACT during PSUM->SBUF eviction (bias = b1 per-partition)
    GEMM2: out[tok,:] += hT[fk, tok-chunk].T @ w2[fk, :] (lhsT = hT, rhs = w2 natural)
    b2 add on DVE during PSUM->SBUF eviction

The x loads are prefetched two macro tiles ahead on the sync (HWDGE)
queue; output stores go out on the gpsimd (SWDGE) queue so an input
load is never queued behind an output store.

FP32r discipline: the BIR verifier requires every producer of an f32r
matmul input to round its output to f32r. xT and the weights are DMA'd
as raw bytes into f32r tiles; hT is produced by ACT gelu writing into
an f32r tile (rounding happens on the write).
"""

import numpy as np

_B, _E, _C, _H, _F = 2, 8, 8192, 512, 2048
_TOK = _B * _C  # 16384 tokens per expert
_P = 128
_T = 512  # tokens per macro tile

_cache = {}


def build_nc(tok=_TOK, act_name="Gelu_apprx_tanh", n_devices=_E,
             loop_reps=1, skip=(), psum_cfg=(5, 3), prefetch=2,
             out_q="gpsimd"):
    import contextlib

    import concourse.mybir as mybir
    import concourse.tile as tile
    from concourse import bacc

    H, F, P, T = _H, _F, _P, _T
    HK, FK = H // P, F // P  # 4, 16 contraction chunks
    NJ = T // P  # 4 token sub-blocks per macro tile
    NM = tok // T  # macro tiles
    f32 = mybir.dt.float32
    f32r = mybir.dt.float32r
    act = getattr(mybir.ActivationFunctionType, act_name)

    nc = bacc.Bacc("TRN2", debug=False, target_bir_lowering=False,
                   num_devices=n_devices)
    # x arrives pre-transposed from the host: [H, tok]
    x = nc.dram_tensor("x", [H, tok], f32r, kind="ExternalInput").ap()
    w1 = nc.dram_tensor("w1", [H, F], f32r, kind="ExternalInput").ap()
    b1 = nc.dram_tensor("b1", [F], f32, kind="ExternalInput").ap()
    w2 = nc.dram_tensor("w2", [F, H], f32r, kind="ExternalInput").ap()
    b2 = nc.dram_tensor("b2", [H], f32, kind="ExternalInput").ap()
    out = nc.dram_tensor("out", [tok, H], f32, kind="ExternalOutput").ap()

    with tile.TileContext(nc) as tc:
        with (
            tc.tile_pool(name="const", bufs=1) as const,
            tc.tile_pool(name="xt", bufs=1 + prefetch) as xt_pool,
            tc.tile_pool(name="ht", bufs=2) as ht_pool,
            tc.tile_pool(name="obuf", bufs=2) as o_pool,
            tc.tile_pool(name="scr", bufs=4) as scratch,
            tc.tile_pool(name="ps1", bufs=psum_cfg[0], space="PSUM") as ps1,
            tc.tile_pool(name="ps2", bufs=psum_cfg[1], space="PSUM") as ps2,
        ):
            # --- weights / constants, resident in SBUF for the whole kernel
            w1_sb = const.tile([P, HK, F], f32r)
            w2_sb = const.tile([P, FK, H], f32r)
            nc.sync.dma_start(
                w1_sb[:], w1.rearrange("(hk p) f -> p hk f", p=P))
            nc.sync.dma_start(
                w2_sb[:], w2.rearrange("(fk p) h -> p fk h", p=P))
            b1_sb = const.tile([P, FK], f32)
            nc.sync.dma_start(b1_sb[:], b1.rearrange("(fk p) -> p fk", p=P))
            b2_row = const.tile([1, H], f32)
            nc.sync.dma_start(b2_row[:], b2[None, :])
            ones = const.tile([1, P], f32)
            nc.any.memset(ones[:], 1.0)
            # broadcast b2 across all 128 partitions via a K=1 matmul
            b2_bc = const.tile([P, H], f32)
            ps_b2 = ps2.tile([P, H], f32, tag="po")
            nc.tensor.matmul(ps_b2[:], ones[:], b2_row[:], start=True, stop=True)
            nc.vector.tensor_copy(b2_bc[:], ps_b2[:])

            def load_x(m):
                """Prefetch macro tile m of xT into SBUF (GEMM1 rhs layout)."""
                xt = xt_pool.tile([P, HK, T], f32r, name=f"xt{m}", tag="xt")
                if "indma" not in skip:
                    nc.sync.dma_start(
                        xt[:],
                        x[:, m * T:(m + 1) * T].rearrange(
                            "(hk p) t -> p hk t", p=P))
                return xt

            def g1_group(xt, ht, m, fm):
                """One GEMM1 accumulation group (4 matmuls) + gelu eviction.

                Evictions alternate engines: even fm goes ACT gelu straight
                from PSUM; odd fm is DVE-copied to SBUF scratch first and
                gelu'd from there. Back-to-back ACT PSUM evictions of
                consecutive groups collapse PE throughput ~17x (measured);
                alternating eliminates that.
                """
                ph = ps1.tile([P, T], f32, name=f"ph{m}_{fm}", tag="ph")
                for hk in range(HK):
                    nc.tensor.matmul(
                        ph[:],
                        w1_sb[:, hk, fm * P:(fm + 1) * P],
                        xt[:, hk, :],
                        start=(hk == 0),
                        stop=(hk == HK - 1),
                    )
                if fm % 2 == 0:
                    nc.scalar.activation(
                        ht[:, fm, :], ph[:], act, bias=b1_sb[:, fm:fm + 1])
                else:
                    sc = scratch.tile([P, T], f32, name=f"sc{m}_{fm}", tag="sc")
                    nc.vector.tensor_copy(sc[:], ph[:])
                    nc.scalar.activation(
                        ht[:, fm, :], sc[:], act, bias=b1_sb[:, fm:fm + 1])

            def g2_group(ht, obig, j):
                """One GEMM2 accumulation group (16 matmuls) + b2 eviction."""
                po = ps2.tile([P, H], f32, tag="po", name="po")
                for fk in range(FK):
                    nc.tensor.matmul(
                        po[:],
                        ht[:, fk, j * P:(j + 1) * P],
                        w2_sb[:, fk, :],
                        start=(fk == 0),
                        stop=(fk == FK - 1),
                    )
                nc.vector.tensor_add(obig[:, j, :], po[:], b2_bc[:])

            def new_ht(m):
                return ht_pool.tile([P, FK, T], f32r, name=f"ht{m}", tag="ht")

            out_dma = {"gpsimd": nc.gpsimd, "sync": nc.sync,
                       "scalar": nc.scalar}[out_q]

            # loop_reps > 1 wraps the body in a hardware loop — used only by
            # the timing harness to amortize per-dispatch overhead.
            reps_ctx = (tc.For_i(0, loop_reps, 1) if loop_reps > 1
                        else contextlib.nullcontext())
            # Software pipeline, one macro ahead on GEMM1, `prefetch` macros
            # ahead on the x DMA:
            #   [dma x(m+2)] ; G1(m+1) x16 ; G2(m) x4 ; out(m)
            # GEMM1(m+1) runs before GEMM2(m) so ACT gelu evictions of ht(m+1)
            # complete under GEMM2(m)'s PE window.
            do_g1 = "gemm1" not in skip
            do_g2 = "gemm2" not in skip
            with reps_ctx:
                xts = {m: load_x(m) for m in range(min(prefetch, NM))}
                ht = new_ht(0)
                if do_g1:
                    for fm in range(FK):
                        g1_group(xts[0], ht, 0, fm)
                for m in range(NM):
                    r = m * T
                    if m + prefetch < NM:
                        xts[m + prefetch] = load_x(m + prefetch)
                    if m + 1 < NM:
                        ht_next = new_ht(m + 1)
                        if do_g1:
                            for fm in range(FK):
                                g1_group(xts[m + 1], ht_next, m + 1, fm)
                    else:
                        ht_next = None
                    xts.pop(m, None)
                    obig = o_pool.tile([P, NJ, H], f32, name=f"ob{m}", tag="ob")
                    if do_g2:
                        for j in range(NJ):
                            g2_group(ht, obig, j)
                        if "outdma" not in skip:
                            out_dma.dma_start(
                                out[r:r + T, :].rearrange(
                                    "(j p) h -> p j h", p=P),
                                obig[:])
                    ht = ht_next

    nc.compile()
    return nc


def kernel(inputs, w1, b1, w2, b2):
    from concourse.bass_utils import run_bass_kernel_spmd

    inputs = np.asarray(inputs, dtype=np.float32)
    w1 = np.asarray(w1, dtype=np.float32)
    b1 = np.asarray(b1, dtype=np.float32)
    w2 = np.asarray(w2, dtype=np.float32)
    b2 = np.asarray(b2, dtype=np.float32)

    B, E, C, H = inputs.shape
    tok = B * C
    # [B, E, C, H] -> per-expert TRANSPOSED token matrix [E, H, B*C]
    xT = np.ascontiguousarray(
        inputs.transpose(1, 3, 0, 2).reshape(E, H, tok))

    if "nc" not in _cache:
        _cache["nc"] = build_nc()
    nc = _cache["nc"]

    in_maps = [
        {
            "x": xT[e],
            "w1": np.ascontiguousarray(w1[e]),
            "b1": np.ascontiguousarray(b1[e]),
            "w2": np.ascontiguousarray(w2[e]),
            "b2": np.ascontiguousarray(b2[e]),
        }
        for e in range(E)
    ]
    res = run_bass_kernel_spmd(nc, in_maps, core_ids=list(range(E)))
    o = np.stack([res.results[e]["out"] for e in range(E)])  # [E, tok, H]
    return np.ascontiguousarray(
        o.reshape(E, B, C, H).transpose(1, 0, 2, 3))


# revision 12
# speedup vs baseline: 1.0173x; 1.0020x over previous
"""Expert-parallel MoE FFN kernel for Trainium2 (8 NeuronCores).

Problem: inputs [B=2, E=8, C=8192, H=512], per-expert FFN
    h   = gelu_tanh(x_e @ w1_e + b1_e)        (w1: [E, H, F=2048])
    out = h @ w2_e + b2_e                     (w2: [E, F, H])

Sharding: expert-parallel — core e owns expert e's tokens [B*C, H] and
weights; no cross-core communication.

Per-core dataflow (matmuls in float32r = full-rate fp32 on the PE):
    x is transposed on the HOST to xT [H, tok] so each macro tile loads
    straight into the GEMM1 rhs layout — no PE transposes at all.
    GEMM1: hT[f,:] += w1[hk, f-chunk].T @ xT[hk, :]   (lhsT = w1, natural layout)
    gelu on ACT during PSUM->SBUF eviction (bias = b1 per-partition)
    GEMM2: out[tok,:] += hT[fk, tok-chunk].T @ w2[fk, :] (lhsT = hT, rhs = w2 natural)
    b2 add on DVE during PSUM->SBUF eviction

The x loads are prefetched two macro tiles ahead on the sync (HWDGE)
queue; output stores go out on the gpsimd (SWDGE) queue so an input
load is never queued behind an output store.

FP32r discipline: the BIR verifier requires every producer of an f32r
matmul input to round its output to f32r. xT and the weights are DMA'd
as raw bytes into f32r tiles; hT is produced by ACT gelu writing into
an f32r tile (rounding happens on the write).
"""

import numpy as np

_B, _E, _C, _H, _F = 2, 8, 8192, 512, 2048
_TOK = _B * _C  # 16384 tokens per expert
_P = 128
_T = 512  # tokens per macro tile

# "bf16" halves nothing on paper (cost model says 1 cyc/row either way)
# but measures materially faster on HW thanks to FWL weight loads; the
# end-to-end rel err vs the fp32 reference is ~3.4e-3 (numpy estimate),
# well inside the 2e-2 gate.
_MM_MODE = "bf16"  # "bf16" | "f32r"

_cache = {}


def build_nc(tok=_TOK, act_name="Gelu_apprx_tanh", n_devices=_E,
             loop_reps=1, skip=(), psum_cfg=(6, 2), prefetch=2,
             out_q="gpsimd", mm_mode=_MM_MODE, unroll=1):
    import contextlib

    import concourse.mybir as mybir
    import concourse.tile as tile
    from concourse import bacc

    H, F, P, T = _H, _F, _P, _T
    HK, FK = H // P, F // P  # 4, 16 contraction chunks
    NJ = T // P  # 4 token sub-blocks per macro tile
    NM = tok // T  # macro tiles
    f32 = mybir.dt.float32
    f32r = mybir.dt.float32r
    mmdt = mybir.dt.bfloat16 if mm_mode == "bf16" else f32r
    act = getattr(mybir.ActivationFunctionType, act_name)

    nc = bacc.Bacc("TRN2", debug=False, target_bir_lowering=False,
                   num_devices=n_devices)
    # x arrives pre-transposed (and pre-cast in bf16 mode) from the host
    x = nc.dram_tensor("x", [H, tok], mmdt, kind="ExternalInput").ap()
    w1 = nc.dram_tensor("w1", [H, F], mmdt, kind="ExternalInput").ap()
    b1 = nc.dram_tensor("b1", [F], f32, kind="ExternalInput").ap()
    w2 = nc.dram_tensor("w2", [F, H], mmdt, kind="ExternalInput").ap()
    b2 = nc.dram_tensor("b2", [H], f32, kind="ExternalInput").ap()
    out = nc.dram_tensor("out", [tok, H], f32, kind="ExternalOutput").ap()

    with tile.TileContext(nc) as tc:
        with (
            tc.tile_pool(name="const", bufs=1) as const,
            tc.tile_pool(name="xt", bufs=1 + prefetch) as xt_pool,
            tc.tile_pool(name="ht", bufs=2) as ht_pool,
            tc.tile_pool(name="obuf", bufs=2) as o_pool,
            tc.tile_pool(name="scr", bufs=4) as scratch,
            tc.tile_pool(name="ps1", bufs=psum_cfg[0], space="PSUM") as ps1,
            tc.tile_pool(name="ps2", bufs=psum_cfg[1], space="PSUM") as ps2,
        ):
            # --- weights / constants, resident in SBUF for the whole kernel.
            # w1 goes out in 4 f-chunks on the sync queue so GEMM1(0) can
            # start after the first chunk + xt(0); everything GEMM1 does
            # NOT need immediately (b-vectors, w2) rides the scalar
            # engine's HWDGE ring instead.
            w1_sb = const.tile([P, HK, F], mmdt)
            w2_sb = const.tile([P, FK, H], mmdt)
            FC = F // 4
            for c in range(4):
                nc.sync.dma_start(
                    w1_sb[:, :, c * FC:(c + 1) * FC],
                    w1[:, c * FC:(c + 1) * FC].rearrange(
                        "(hk p) f -> p hk f", p=P))
            b1_sb = const.tile([P, FK], f32)
            nc.scalar.dma_start(b1_sb[:], b1.rearrange("(fk p) -> p fk", p=P))
            b2_row = const.tile([1, H], f32)
            nc.scalar.dma_start(b2_row[:], b2[None, :])
            nc.scalar.dma_start(
                w2_sb[:], w2.rearrange("(fk p) h -> p fk h", p=P))
            ones = const.tile([1, P], f32)
            nc.any.memset(ones[:], 1.0)
            # broadcast b2 across all 128 partitions via a K=1 matmul
            b2_bc = const.tile([P, H], f32)
            ps_b2 = ps2.tile([P, H], f32, tag="po")
            nc.tensor.matmul(ps_b2[:], ones[:], b2_row[:], start=True, stop=True)
            nc.vector.tensor_copy(b2_bc[:], ps_b2[:])

            def load_x(key, m):
                """Prefetch macro tile m of xT into SBUF (GEMM1 rhs layout)."""
                xt = xt_pool.tile([P, HK, T], mmdt, name=f"xt{key}", tag="xt")
                if "indma" not in skip:
                    nc.sync.dma_start(
                        xt[:],
                        x[:, m * T:(m + 1) * T].rearrange(
                            "(hk p) t -> p hk t", p=P))
                return xt

            def g1_group(xt, ht, m, fm):
                """One GEMM1 accumulation group (4 matmuls) + gelu eviction.

                Evictions alternate engines: even fm goes ACT gelu straight
                from PSUM; odd fm is DVE-copied to SBUF scratch first and
                gelu'd from there. Back-to-back ACT PSUM evictions of
                consecutive groups collapse PE throughput ~17x (measured);
                alternating eliminates that.
                """
                ph = ps1.tile([P, T], f32, name=f"ph{m}_{fm}", tag="ph")
                for hk in range(HK):
                    nc.tensor.matmul(
                        ph[:],
                        w1_sb[:, hk, fm * P:(fm + 1) * P],
                        xt[:, hk, :],
                        start=(hk == 0),
                        stop=(hk == HK - 1),
                    )
                if fm % 2 == 0:
                    nc.scalar.activation(
                        ht[:, fm, :], ph[:], act, bias=b1_sb[:, fm:fm + 1])
                else:
                    sc = scratch.tile([P, T], f32, name=f"sc{m}_{fm}", tag="sc")
                    nc.vector.tensor_copy(sc[:], ph[:])
                    nc.scalar.activation(
                        ht[:, fm, :], sc[:], act, bias=b1_sb[:, fm:fm + 1])

            def g2_group(ht, obig, j):
                """One GEMM2 accumulation group (16 matmuls) + b2 eviction."""
                po = ps2.tile([P, H], f32, tag="po", name="po")
                for fk in range(FK):
                    nc.tensor.matmul(
                        po[:],
                        ht[:, fk, j * P:(j + 1) * P],
                        w2_sb[:, fk, :],
                        start=(fk == 0),
                        stop=(fk == FK - 1),
                    )
                nc.vector.tensor_add(obig[:, j, :], po[:], b2_bc[:])

            def new_ht(key):
                return ht_pool.tile([P, FK, T], mmdt,
                                    name=f"ht{key}", tag="ht")

            out_dma = {"gpsimd": nc.gpsimd, "sync": nc.sync,
                       "scalar": nc.scalar}[out_q]

            # loop_reps > 1 wraps the body in a hardware loop — used only by
            # the timing harness to amortize per-dispatch overhead. For_i
            # inserts an all-engine barrier per iteration, so `unroll`
            # repeats the body within one iteration to keep the measured
            # slope closer to the barrier-free steady state.
            assert loop_reps % unroll == 0
            n_iters = loop_reps // unroll
            reps_ctx = (tc.For_i(0, n_iters, 1) if n_iters > 1
                        else contextlib.nullcontext())
            # Software pipeline, one macro ahead on GEMM1, `prefetch` macros
            # ahead on the x DMA:
            #   [dma x(m+2)] ; G1(m+1) x16 ; G2(m) x4 ; out(m)
            # GEMM1(m+1) runs before GEMM2(m) so ACT gelu evictions of ht(m+1)
            # complete under GEMM2(m)'s PE window.
            do_g1 = "gemm1" not in skip
            do_g2 = "gemm2" not in skip

            def emit_body(rep):
                xts = {m: load_x(f"{rep}_{m}", m)
                       for m in range(min(prefetch, NM))}
                ht = new_ht(f"{rep}_0")
                if do_g1:
                    for fm in range(FK):
                        g1_group(xts[0], ht, f"{rep}_0", fm)
                for m in range(NM):
                    r = m * T
                    if m + prefetch < NM:
                        xts[m + prefetch] = load_x(
                            f"{rep}_{m + prefetch}", m + prefetch)
                    if m + 1 < NM:
                        ht_next = new_ht(f"{rep}_{m + 1}")
                        if do_g1:
                            for fm in range(FK):
                                g1_group(xts[m + 1], ht_next,
                                         f"{rep}_{m + 1}", fm)
                    else:
                        ht_next = None
                    xts.pop(m, None)
                    obig = o_pool.tile([P, NJ, H], f32,
                                       name=f"ob{rep}_{m}", tag="ob")
                    if do_g2:
                        for j in range(NJ):
                            g2_group(ht, obig, j)
                        if "outdma" not in skip:
                            out_dma.dma_start(
                                out[r:r + T, :].rearrange(
                                    "(j p) h -> p j h", p=P),
                                obig[:])
                    ht = ht_next

            with reps_ctx:
                for rep in range(unroll):
                    emit_body(rep)

    nc.compile()
    return nc


def kernel(inputs, w1, b1, w2, b2):
    from concourse.bass_utils import run_bass_kernel_spmd

    inputs = np.asarray(inputs, dtype=np.float32)
    w1 = np.asarray(w1, dtype=np.float32)
    b1 = np.asarray(b1, dtype=np.float32)
    w2 = np.asarray(w2, dtype=np.float32)
    b2 = np.asarray(b2, dtype=np.float32)

    B, E, C, H = inputs.shape
    tok = B * C
    # [B, E, C, H] -> per-expert TRANSPOSED token matrix [E, H, B*C]
    xT = np.ascontiguousarray(
        inputs.transpose(1, 3, 0, 2).reshape(E, H, tok))
    if _MM_MODE == "bf16":
        import ml_dtypes
        bf16 = ml_dtypes.bfloat16
        xT, w1, w2 = xT.astype(bf16), w1.astype(bf16), w2.astype(bf16)

    if "nc" not in _cache:
        _cache["nc"] = build_nc()
    nc = _cache["nc"]

    in_maps = [
        {
            "x": xT[e],
            "w1": np.ascontiguousarray(w1[e]),
            "b1": np.ascontiguousarray(b1[e]),
            "w2": np.ascontiguousarray(w2[e]),
            "b2": np.ascontiguousarray(b2[e]),
        }
        for e in range(E)
    ]
    res = run_bass_kernel_spmd(nc, in_maps, core_ids=list(range(E)))
    o = np.stack([res.results[e]["out"] for e in range(E)])  # [E, tok, H]
    return np.ascontiguousarray(
        o.reshape(E, B, C, H).transpose(1, 0, 2, 3))
